# revision 7
# baseline (speedup 1.0000x reference)
"""ChildSum TreeLSTM cell on 8 Trainium2 NeuronCores (Bass/Tile).

Strategy (graph-parallel, per the sharding hint):
  - Partition nodes (parents) into 8 contiguous ranges of N/8; each core owns
    the segment-sum + cell update for its parents.
  - Host does INDEX prep only: sort edges by parent, bucket them into
    512-parent blocks, pad each block to a uniform number of 128-edge chunks
    (SPMD: one program, 8 cores), build a per-core compacted child table
    (halo nodes) and remapped gather indices, pre-transpose x.
  - Device does all the FLOP/memory work: indirect-DMA gather of child h||c
    rows, one-hot segment-sum via PE matmuls accumulated in PSUM, dense LSTM
    matmuls, sigmoid/tanh elementwise, output store.

Layouts per 512-parent block (s = local parent slot, e = edge, h = feature):
  V[e, 0:128]=h_child, V[e,128:256]=c_child   (gathered, 128 edges/chunk)
  P[e, s] = one-hot(slot[e])                  (built on-device via iota+is_equal)
  h_sumT[h, s] += V_h^T P    (PE, PSUM)       -> lhsT for U_* matmuls
  c_sum [s, h] += P^T V_c    (PE, PSUM, 4 col-slices)  -> elementwise operand
  f/iou [n, h] = xT^T W + h_sumT^T U          (PE, xT pre-transposed on host)
"""

import os
import sys
import time

for _p in ("/opt/trn_rl_repo", "/root/.axon_site/_ro/trn_rl_repo"):
    if os.path.isdir(_p) and _p not in sys.path:
        sys.path.insert(0, _p)

import numpy as np

import concourse.bass as bass
import concourse.tile as tile
from concourse import mybir
from concourse.bass_utils import run_bass_kernel_spmd
from concourse.vector_clock import ScopedClock

CORES = 8
S = 512          # parents per block (= PSUM bank free dim in fp32)
P128 = 128

F32 = mybir.dt.float32
I32 = mybir.dt.int32
AF = mybir.ActivationFunctionType
ALU = mybir.AluOpType

# ---------------------------------------------------------------------------
# Workarounds: the walrus build in this container accepts at most ONE sync
# wait per instruction. (a) chunk the Tile tail-drain waits onto nops;
# (b) post-pass that hoists extra waits of any instruction onto preceding
# NoOps on the same engine.
# ---------------------------------------------------------------------------

def _drain_and_barrier_chunked(self, tick_clock, wait_clock):
    probe = self.nc.sync.nop()
    wait_clock.add_sem_waits(probe.ins, ScopedClock({None: tick_clock.global_clock}))
    si = probe.ins.sync_info
    waits = list(si.on_wait)
    probe.ins.sync_info = mybir.SyncInfo(on_wait=waits[:1], on_update=list(si.on_update))
    for i in range(1, len(waits)):
        nop = self.nc.sync.nop()
        nop.ins.sync_info = mybir.SyncInfo(on_wait=waits[i:i + 1], on_update=[])
    self.nc.sync.drain()
    self.nc.all_engine_barrier()
    popped = self.nc._tile_sem_poison_stack.pop()
    assert popped is self._sem_poison
    self.nc.clear_and_free_semaphores(list(self.sems.allocated().values()))
    self.nc.all_engine_barrier()


tile.TileContext._drain_and_barrier = _drain_and_barrier_chunked

_WSPLIT_CTR = [0]


def _split_multi_waits(nc):
    n_split = 0
    for f in nc.m.functions:
        for bb in f.blocks:
            insts = list(bb.instructions)
            if not any(
                i.sync_info is not None and i.sync_info.on_wait and len(i.sync_info.on_wait) > 1
                for i in insts
            ):
                continue
            new = []
            for inst in insts:
                si = inst.sync_info
                if si is not None and si.on_wait and len(si.on_wait) > 1:
                    waits = list(si.on_wait)
                    n_split += 1
                    for w in waits[:-1]:
                        _WSPLIT_CTR[0] += 1
                        new.append(
                            mybir.InstNoOp(
                                name=f"I-wsplit-{_WSPLIT_CTR[0]}",
                                engine=inst.engine,
                                debug=inst.debug,
                                ins=[],
                                outs=[],
                                sync_info=mybir.SyncInfo(on_wait=[w], on_update=[]),
                            )
                        )
                    inst.sync_info = mybir.SyncInfo(
                        on_wait=[waits[-1]], on_update=list(si.on_update)
                    )
                new.append(inst)
            bb.instructions = new
    return n_split


# ---------------------------------------------------------------------------
# Host-side index prep
# ---------------------------------------------------------------------------

def _prep(x, h, c, child_idx, parent_idx):
    N = x.shape[0]
    E = child_idx.shape[0]
    npc = (N + CORES - 1) // CORES            # parents per core
    nb = (npc + S - 1) // S                   # blocks per core
    npad = nb * S

    order = np.argsort(parent_idx, kind="stable")
    sp = np.asarray(parent_idx)[order]
    sc = np.asarray(child_idx)[order]

    # per-(core, block) edge ranges; uniform chunk count C_max across all
    bounds = np.empty((CORES, nb + 1), np.int64)
    for i in range(CORES):
        base = i * npc
        edges_hi = np.minimum(base + np.arange(nb + 1) * S, min(base + npc, N))
        bounds[i] = np.searchsorted(sp, edges_hi, "left")
    m = np.diff(bounds, axis=1)
    c_max = max(1, int(np.ceil(m.max() / P128)))
    nch = nb * c_max                          # chunks per core

    hc = np.ascontiguousarray(
        np.concatenate([np.asarray(h), np.asarray(c)], axis=1), dtype=np.float32
    )

    per_core = []
    u_sizes = []
    for i in range(CORES):
        base = i * npc
        slots = np.full(nch * P128, -1.0, np.float32)
        gidx = np.zeros(nch * P128, np.int64)
        for b in range(nb):
            e0, e1 = bounds[i, b], bounds[i, b + 1]
            mm = e1 - e0
            if mm == 0:
                continue
            ch = sc[e0:e1]
            sl = (sp[e0:e1] - base - b * S).astype(np.float32)
            so = np.argsort(ch, kind="stable")   # gather locality
            off = b * c_max * P128
            slots[off:off + mm] = sl[so]
            gidx[off:off + mm] = ch[so]
        real = slots >= 0
        uniq = np.unique(gidx[real]) if real.any() else np.array([0], np.int64)
        gr = np.searchsorted(uniq, gidx)
        gr[~real] = 0
        u_sizes.append(len(uniq))
        per_core.append((slots, gr, uniq))

    u_max = max(u_sizes)

    in_maps = []
    for i in range(CORES):
        slots, gr, uniq = per_core[i]
        base = i * npc
        hc_sub = np.zeros((u_max, 256), np.float32)
        hc_sub[: len(uniq)] = hc[uniq]
        xT = np.zeros((P128, npad), np.float32)
        hi = min(base + npc, N)
        xT[:, : hi - base] = np.asarray(x)[base:hi].T
        in_maps.append(
            {
                "xT": xT,
                "hc": hc_sub,
                "slots": np.ascontiguousarray(slots.reshape(nch, P128).T),
                "gidx": np.ascontiguousarray(gr.reshape(nch, P128).T).astype(np.int32),
            }
        )
    return in_maps, npc, nb, npad, c_max, u_max


# ---------------------------------------------------------------------------
# Device program
# ---------------------------------------------------------------------------

def _build_nc(nb, npad, c_max, u_max, W_f, U_f, b_f, W_iou, U_iou, b_iou):
    nch = nb * c_max
    with_bias = bool(np.any(b_f)) or bool(np.any(b_iou))

    nc = bass.Bass("TRN2", target_bir_lowering=False, debug=False)
    xT_t = nc.dram_tensor("xT", [P128, npad], F32, kind="ExternalInput")
    hc_t = nc.dram_tensor("hc", [u_max, 256], F32, kind="ExternalInput")
    slots_t = nc.dram_tensor("slots", [P128, nch], F32, kind="ExternalInput")
    gidx_t = nc.dram_tensor("gidx", [P128, nch], I32, kind="ExternalInput")
    wf_t = nc.dram_tensor("W_f", [128, 128], F32, kind="ExternalInput")
    uf_t = nc.dram_tensor("U_f", [128, 128], F32, kind="ExternalInput")
    wio_t = nc.dram_tensor("W_iou", [128, 384], F32, kind="ExternalInput")
    uio_t = nc.dram_tensor("U_iou", [128, 384], F32, kind="ExternalInput")
    if with_bias:
        bf_t = nc.dram_tensor("b_f", [1, 128], F32, kind="ExternalInput")
        bio_t = nc.dram_tensor("b_iou", [1, 384], F32, kind="ExternalInput")
    out_t = nc.dram_tensor("out", [npad, 256], F32, kind="ExternalOutput")

    with tile.TileContext(nc) as tc:
        with (
            tc.tile_pool(name="const", bufs=1) as cpool,
            tc.tile_pool(name="vpool", bufs=c_max + 3) as vpool,
            tc.tile_pool(name="ppool", bufs=c_max + 2) as ppool,
            tc.tile_pool(name="xpool", bufs=3) as xpool,
            tc.tile_pool(name="hpool", bufs=2) as hpool,
            tc.tile_pool(name="fpool", bufs=2) as fpool,
            tc.tile_pool(name="gpool", bufs=2) as gpool,
            tc.tile_pool(name="iopool", bufs=3) as iopool,
            tc.tile_pool(name="upool", bufs=3) as upool,
            tc.tile_pool(name="tpool", bufs=3) as tpool,
            tc.tile_pool(name="opool", bufs=4) as opool,
            tc.tile_pool(name="psA", bufs=2, space="PSUM") as psA,
            tc.tile_pool(name="psB", bufs=2, space="PSUM") as psB,
            tc.tile_pool(name="psF", bufs=2, space="PSUM") as psF,
            tc.tile_pool(name="psIO", bufs=2, space="PSUM") as psIO,
        ):
            wf_sb = cpool.tile([128, 128], F32)
            nc.sync.dma_start(out=wf_sb[:], in_=wf_t[:, :])
            uf_sb = cpool.tile([128, 128], F32)
            nc.sync.dma_start(out=uf_sb[:], in_=uf_t[:, :])
            wio_sb = cpool.tile([128, 384], F32)
            nc.sync.dma_start(out=wio_sb[:], in_=wio_t[:, :])
            uio_sb = cpool.tile([128, 384], F32)
            nc.sync.dma_start(out=uio_sb[:], in_=uio_t[:, :])
            slots_sb = cpool.tile([P128, nch], F32)
            nc.sync.dma_start(out=slots_sb[:], in_=slots_t[:, :])
            gidx_sb = cpool.tile([P128, nch], I32)
            nc.sync.dma_start(out=gidx_sb[:], in_=gidx_t[:, :])
            iota_sb = cpool.tile([128, S], F32)
            nc.gpsimd.iota(
                iota_sb[:], [[1, S]], channel_multiplier=0,
                allow_small_or_imprecise_dtypes=True,
            )
            if with_bias:
                bf_row = cpool.tile([1, 128], F32)
                nc.sync.dma_start(out=bf_row[:], in_=bf_t[:, :])
                bio_row = cpool.tile([1, 384], F32)
                nc.sync.dma_start(out=bio_row[:], in_=bio_t[:, :])
                bf_sb = cpool.tile([128, S], F32)
                bio_sb = cpool.tile([128, 384], F32)
                bf_one = cpool.tile([128, 128], F32)
                nc.gpsimd.partition_broadcast(bf_one[:], bf_row[:])
                nc.gpsimd.partition_broadcast(bio_sb[:], bio_row[:])
                for jj in range(4):
                    nc.vector.tensor_copy(out=bf_sb[:, jj * 128:(jj + 1) * 128], in_=bf_one[:])

            for b in range(nb):
                xT_sb = xpool.tile([128, S], F32)
                nc.sync.dma_start(out=xT_sb[:], in_=xT_t[:, b * S:(b + 1) * S])

                ps_hT = psA.tile([128, S], F32, space="PSUM")
                ps_c = psB.tile([128, S], F32, space="PSUM")
                # NB: matmuls of one PSUM accumulation group must be emitted
                # consecutively — interleaving groups on the same bank
                # produced wrong results on HW. Gather all chunks first.
                last = c_max - 1
                Vs, Ps = [], []
                for k in range(c_max):
                    j = b * c_max + k
                    V = vpool.tile([128, 256], F32)
                    nc.gpsimd.indirect_dma_start(
                        out=V[:], out_offset=None, in_=hc_t[:],
                        in_offset=bass.IndirectOffsetOnAxis(ap=gidx_sb[:, j:j + 1], axis=0),
                    )
                    P = ppool.tile([128, S], F32)
                    nc.vector.tensor_scalar(
                        out=P[:], in0=iota_sb[:], scalar1=slots_sb[:, j:j + 1],
                        scalar2=None, op0=ALU.is_equal,
                    )
                    Vs.append(V)
                    Ps.append(P)
                for k in range(c_max):
                    nc.tensor.matmul(
                        out=ps_hT[:], lhsT=Vs[k][:, 0:128], rhs=Ps[k][:],
                        start=(k == 0), stop=(k == last),
                    )
                for jj in range(4):
                    for k in range(c_max):
                        nc.tensor.matmul(
                            out=ps_c[:, jj * 128:(jj + 1) * 128],
                            lhsT=Ps[k][:, jj * 128:(jj + 1) * 128], rhs=Vs[k][:, 128:256],
                            start=(k == 0), stop=(k == last),
                        )

                hsumT_sb = hpool.tile([128, S], F32)
                nc.vector.tensor_copy(out=hsumT_sb[:], in_=ps_hT[:])

                ps_f = psF.tile([128, S], F32, space="PSUM")
                for jj in range(4):
                    sl = slice(jj * 128, (jj + 1) * 128)
                    nc.tensor.matmul(out=ps_f[:, sl], lhsT=xT_sb[:, sl], rhs=wf_sb[:],
                                     start=True, stop=False)
                    nc.tensor.matmul(out=ps_f[:, sl], lhsT=hsumT_sb[:, sl], rhs=uf_sb[:],
                                     start=False, stop=True)
                if with_bias:
                    nc.vector.tensor_tensor(out=ps_f[:], in0=ps_f[:], in1=bf_sb[:], op=ALU.add)
                fsig = fpool.tile([128, S], F32)
                nc.scalar.activation(out=fsig[:], in_=ps_f[:], func=AF.Sigmoid)
                cagg = gpool.tile([128, S], F32)
                nc.vector.tensor_tensor(out=cagg[:], in0=fsig[:], in1=ps_c[:], op=ALU.mult)

                for jj in range(4):
                    sl = slice(jj * 128, (jj + 1) * 128)
                    ps_io = psIO.tile([128, 384], F32, space="PSUM")
                    nc.tensor.matmul(out=ps_io[:], lhsT=xT_sb[:, sl], rhs=wio_sb[:],
                                     start=True, stop=False)
                    nc.tensor.matmul(out=ps_io[:], lhsT=hsumT_sb[:, sl], rhs=uio_sb[:],
                                     start=False, stop=True)
                    if with_bias:
                        nc.vector.tensor_tensor(out=ps_io[:], in0=ps_io[:], in1=bio_sb[:], op=ALU.add)
                    iosig = iopool.tile([128, 256], F32)
                    nc.scalar.activation(out=iosig[:], in_=ps_io[:, 0:256], func=AF.Sigmoid)
                    utanh = upool.tile([128, 128], F32)
                    nc.scalar.activation(out=utanh[:], in_=ps_io[:, 256:384], func=AF.Tanh)
                    outsb = opool.tile([128, 256], F32)
                    nc.vector.tensor_tensor(out=outsb[:, 128:256], in0=iosig[:, 0:128],
                                            in1=utanh[:], op=ALU.mult)
                    nc.vector.tensor_tensor(out=outsb[:, 128:256], in0=outsb[:, 128:256],
                                            in1=cagg[:, sl], op=ALU.add)
                    tanhc = tpool.tile([128, 128], F32)
                    nc.scalar.activation(out=tanhc[:], in_=outsb[:, 128:256], func=AF.Tanh)
                    nc.vector.tensor_tensor(out=outsb[:, 0:128], in0=iosig[:, 128:256],
                                            in1=tanhc[:], op=ALU.mult)
                    nc.sync.dma_start(
                        out=out_t[b * S + jj * 128: b * S + (jj + 1) * 128, :],
                        in_=outsb[:],
                    )

    _split_multi_waits(nc)
    return nc, with_bias


LAST_EXEC_TIME_NS = None
LAST_RESULTS = None
_LAST_RUN = None  # (nc, in_maps) for benchmarking


def _timed_run(nc, in_maps, iters=20, warmup=3):
    """Re-execute the compiled NEFF with device-resident inputs, pipelined via
    jax async dispatch; returns mean per-iteration wall ns (upper bound on HW
    exec time; NTFF profiling hook is unavailable in this container)."""
    import jax
    import jax.numpy as jnp
    from jax.sharding import Mesh, PartitionSpec, NamedSharding
    try:
        from jax.experimental.shard_map import shard_map
    except ImportError:
        from jax.shard_map import shard_map
    from concourse import bass2jax

    bass2jax.install_neuronx_cc_hook()
    n_cores = len(in_maps)

    partition_name = nc.partition_id_tensor.name if nc.partition_id_tensor else None
    in_names, out_names, out_avals, zero_outs = [], [], [], []
    for alloc in nc.m.functions[0].allocations:
        if not isinstance(alloc, mybir.MemoryLocationSet):
            continue
        name = alloc.memorylocations[0].name
        if alloc.kind == "ExternalInput":
            if name != partition_name:
                in_names.append(name)
        elif alloc.kind == "ExternalOutput":
            shape = tuple(alloc.tensor_shape)
            dtype = mybir.dt.np(alloc.dtype)
            out_names.append(name)
            out_avals.append(jax.core.ShapedArray(shape, dtype))
            zero_outs.append(np.zeros(shape, dtype))
    n_params = len(in_names)
    all_names = in_names + out_names
    if partition_name is not None:
        all_names = all_names + [partition_name]

    def _body(*args):
        operands = list(args)
        if partition_name is not None:
            operands.append(bass2jax.partition_id_tensor())
        outs = bass2jax._bass_exec_p.bind(
            *operands,
            out_avals=tuple(out_avals),
            in_names=tuple(all_names),
            out_names=tuple(out_names),
            lowering_input_output_aliases=(),
            sim_require_finite=True,
            sim_require_nnan=True,
            nc=nc,
        )
        return tuple(outs)

    devices = jax.devices()[:n_cores]
    mesh = Mesh(np.asarray(devices), ("core",))
    spec = PartitionSpec("core")
    fn = jax.jit(
        shard_map(
            _body, mesh=mesh,
            in_specs=(spec,) * (n_params + len(out_names)),
            out_specs=(spec,) * len(out_names),
            check_rep=False,
        ),
        keep_unused=True,
    )
    sh = NamedSharding(mesh, spec)
    args = [
        jax.device_put(
            np.concatenate([np.asarray(in_maps[c][nm]) for c in range(n_cores)], axis=0), sh
        )
        for nm in in_names
    ] + [
        jax.device_put(np.concatenate([z] * n_cores, axis=0), sh) for z in zero_outs
    ]

    for _ in range(warmup):
        out = fn(*args)
    jax.block_until_ready(out)
    t0 = time.perf_counter()
    outs = [fn(*args) for _ in range(iters)]
    jax.block_until_ready(outs)
    dt = time.perf_counter() - t0
    return int(dt / iters * 1e9)


def benchmark_last(iters=20):
    global LAST_EXEC_TIME_NS
    assert _LAST_RUN is not None, "call kernel() first"
    nc, in_maps = _LAST_RUN
    LAST_EXEC_TIME_NS = _timed_run(nc, in_maps, iters=iters)
    return LAST_EXEC_TIME_NS


def kernel(x, h, c, child_idx, parent_idx, W_f, U_f, b_f, W_iou, U_iou, b_iou,
           trace=False, trace_cores=None):
    global LAST_EXEC_TIME_NS, LAST_RESULTS
    x = np.asarray(x, np.float32)
    N = x.shape[0]
    in_maps, npc, nb, npad, c_max, u_max = _prep(x, h, c, child_idx, parent_idx)

    nc, with_bias = _build_nc(
        nb, npad, c_max, u_max, W_f, U_f, b_f, W_iou, U_iou, b_iou
    )
    for im in in_maps:
        im["W_f"] = np.asarray(W_f, np.float32)
        im["U_f"] = np.asarray(U_f, np.float32)
        im["W_iou"] = np.asarray(W_iou, np.float32)
        im["U_iou"] = np.asarray(U_iou, np.float32)
        if with_bias:
            im["b_f"] = np.asarray(b_f, np.float32).reshape(1, 128)
            im["b_iou"] = np.asarray(b_iou, np.float32).reshape(1, 384)

    kwargs = {}
    if trace:
        kwargs["trace"] = True
        if trace_cores is not None:
            kwargs["trace_cores"] = trace_cores
    res = run_bass_kernel_spmd(nc, in_maps, core_ids=list(range(CORES)), **kwargs)
    LAST_EXEC_TIME_NS = res.exec_time_ns
    LAST_RESULTS = res
    global _LAST_RUN
    _LAST_RUN = (nc, in_maps)

    parts = []
    for i in range(CORES):
        lo = i * npc
        hi = min(lo + npc, N)
        parts.append(res.results[i]["out"][: hi - lo])
    return np.concatenate(parts, axis=0)
